# revision 4
# baseline (speedup 1.0000x reference)
"""PointNet-style encoder (conv1x1 stack + ragged segment-max) on 8 Trainium2 cores.

Strategy
--------
* BN folded into the conv weights host-side; every layer becomes matmul+bias+ReLU.
* Feature-major on device: activations live as [C, points] tiles, points stream
  through the PE as the matmul free dimension in 512-point macro-tiles.
* Raggedness handled host-side: each segment's points padded to a multiple of
  512 by duplicating its own points (exact under max-pooling), so every
  macro-tile belongs to exactly one segment. Segments are bin-packed 4 per
  core, and every core is padded to the same macro-tile count Tm with
  duplicate tiles, so a single SPMD program covers all cores.
* The mid-network segment-max feeds back via the concat identity
  concat(f2, g) @ W3 = f2 @ W3[:256] + g @ W3[256:]; the g-contribution plus
  b3 becomes a per-macro-tile bias column (table U), applied by the activation
  instruction's per-partition bias operand.
* Per-macro-tile layer-4 maxes are returned raw; the host applies
  relu(. + b4) and the per-segment max over tiles (monotonicity makes this
  exact).
* Matmuls run in float16 (fp32 PSUM accumulate): 1 cycle/column, ~1e-3 rel err.
  (float32r measured 2 cycles/column on HW: it is a 2-pass replicated mode.)
"""

import numpy as np

import concourse.bass as bass
import concourse.mybir as mybir
import concourse.tile as tile
from concourse import bacc
from concourse.bass_utils import run_bass_kernel_spmd

N_CORES = 8
PT = 512  # points per macro-tile
EPS = 1e-3  # keras BatchNormalization default epsilon

F32 = mybir.dt.float32
F16 = mybir.dt.float16
AF = mybir.ActivationFunctionType
AXX = mybir.AxisListType.X
ALU_MAX = mybir.AluOpType.max

_PROGRAM_CACHE: dict = {}


def _build_program(Tm: int, S: int):
    """One SPMD program for all cores: Tm macro-tiles, S segment slots."""
    nc = bacc.Bacc("TRN2")
    Tmp = Tm + (Tm % 2)  # fp32r matmul needs an even moving free dim

    xT = nc.dram_tensor("xT", [3, Tm * PT], F16, kind="ExternalInput")
    mask = nc.dram_tensor("mask", [128, S, Tmp], F32, kind="ExternalInput")
    w1 = nc.dram_tensor("w1", [3, 128], F16, kind="ExternalInput")
    w2 = nc.dram_tensor("w2", [128, 2, 128], F16, kind="ExternalInput")
    w3a = nc.dram_tensor("w3a", [128, 2, 4, 128], F16, kind="ExternalInput")
    w3b = nc.dram_tensor("w3b", [128, 2, 4, 128], F16, kind="ExternalInput")
    w4 = nc.dram_tensor("w4", [128, 4, 8, 128], F16, kind="ExternalInput")
    b1 = nc.dram_tensor("b1", [128, 1], F32, kind="ExternalInput")
    b2 = nc.dram_tensor("b2", [128, 2], F32, kind="ExternalInput")
    b3 = nc.dram_tensor("b3", [128, 4], F32, kind="ExternalInput")
    mx4 = nc.dram_tensor("mx4", [128, 8, Tm], F32, kind="ExternalOutput")

    with tile.TileContext(nc) as tc:
        with (
            tc.tile_pool(name="const", bufs=1) as constp,
            tc.tile_pool(name="xp", bufs=4) as xp,
            tc.tile_pool(name="h1p", bufs=3) as h1p,
            tc.tile_pool(name="f2p", bufs=2) as f2p,
            tc.tile_pool(name="h3p", bufs=2) as h3p,
            tc.tile_pool(name="tmpp", bufs=4) as tmpp,
            tc.tile_pool(name="ps", bufs=8, space="PSUM") as ps,
        ):
            w1_sb = constp.tile([3, 128], F16)
            nc.sync.dma_start(out=w1_sb, in_=w1.ap())
            w2_sb = constp.tile([128, 2, 128], F16)
            nc.sync.dma_start(out=w2_sb, in_=w2.ap())
            w3a_sb = constp.tile([128, 2, 4, 128], F16)
            nc.sync.dma_start(out=w3a_sb, in_=w3a.ap())
            w3b_sb = constp.tile([128, 2, 4, 128], F16)
            nc.sync.dma_start(out=w3b_sb, in_=w3b.ap())
            w4_sb = constp.tile([128, 4, 8, 128], F16)
            nc.sync.dma_start(out=w4_sb, in_=w4.ap())
            b1_sb = constp.tile([128, 1], F32)
            nc.sync.dma_start(out=b1_sb, in_=b1.ap())
            b2_sb = constp.tile([128, 2], F32)
            nc.sync.dma_start(out=b2_sb, in_=b2.ap())
            b3_sb = constp.tile([128, 4], F32)
            nc.sync.dma_start(out=b3_sb, in_=b3.ap())
            mask_sb = constp.tile([128, S, Tmp], F32)
            nc.sync.dma_start(out=mask_sb, in_=mask.ap())

            Mx2_sb = constp.tile([128, 2, Tm], F32)
            Mx2r_sb = constp.tile([128, 2, Tm], F32)
            g_sb = constp.tile([128, 2, S], F32)
            Gacc_sb = constp.tile([128, 2, Tmp], F32)
            G2_sb = constp.tile([128, 2, Tmp], F16)
            U_sb = constp.tile([128, 4, Tmp], F32)
            Mx4_sb = constp.tile([128, 8, Tm], F32)

            xTr = xT.ap()

            # ---- pass 1: per-macro-tile max of the layer-2 preactivation ----
            for t in range(Tm):
                x_sb = xp.tile([3, PT], F16, tag="x", name=f"x1_{t}")
                nc.sync.dma_start(out=x_sb, in_=xTr[:, t * PT : (t + 1) * PT])
                ps1 = ps.tile([128, PT], F32, tag="ps", name=f"ps1a_{t}")
                nc.tensor.matmul(ps1[:, :], w1_sb[:, :], x_sb[:, :], start=True, stop=True)
                h1_sb = h1p.tile([128, PT], F16, tag="h1", name=f"h1a_{t}")
                nc.scalar.activation(out=h1_sb, in_=ps1, func=AF.Relu, bias=b1_sb[:, 0:1])
                for c in range(2):
                    ps2 = ps.tile([128, PT], F32, tag="ps", name=f"ps2a_{t}_{c}")
                    nc.tensor.matmul(ps2[:, :], w2_sb[:, c, :], h1_sb[:, :], start=True, stop=True)
                    nc.vector.tensor_reduce(
                        out=Mx2_sb[:, c, t : t + 1], in_=ps2, axis=AXX, op=ALU_MAX
                    )

            # ---- g assembly: per-slot max via host-provided 0/1 masks ----
            for c in range(2):
                nc.scalar.activation(
                    out=Mx2r_sb[:, c, :], in_=Mx2_sb[:, c, :], func=AF.Relu, bias=b2_sb[:, c : c + 1]
                )
            for c in range(2):
                for s in range(S):
                    tmp = tmpp.tile([128, Tm], F32, tag="tmp", name=f"tmpg_{c}_{s}")
                    nc.vector.tensor_mul(tmp, Mx2r_sb[:, c, :], mask_sb[:, s, :Tm])
                    nc.vector.tensor_reduce(
                        out=g_sb[:, c, s : s + 1], in_=tmp, axis=AXX, op=ALU_MAX
                    )
            # expand g back to a per-tile table G2[:, c, t] = g[:, c, slot(t)]
            for c in range(2):
                nc.vector.tensor_scalar_mul(Gacc_sb[:, c, :], mask_sb[:, 0, :], g_sb[:, c, 0:1])
                for s in range(1, S):
                    tmp2 = tmpp.tile([128, Tmp], F32, tag="tmp2", name=f"tmpe_{c}_{s}")
                    nc.vector.tensor_scalar_mul(tmp2, mask_sb[:, s, :], g_sb[:, c, s : s + 1])
                    nc.vector.tensor_add(Gacc_sb[:, c, :], Gacc_sb[:, c, :], tmp2)
                nc.scalar.copy(G2_sb[:, c, :], Gacc_sb[:, c, :])
            # U[:, m, t] = (W3b.T @ G2)[:, t] + b3  — per-tile bias column for L3
            for m in range(4):
                psu = ps.tile([128, PT], F32, tag="ps", name=f"psu_{m}")
                nc.tensor.matmul(psu[:, :Tmp], w3b_sb[:, 0, m, :], G2_sb[:, 0, :], start=True, stop=False)
                nc.tensor.matmul(psu[:, :Tmp], w3b_sb[:, 1, m, :], G2_sb[:, 1, :], start=False, stop=True)
                nc.scalar.activation(
                    out=U_sb[:, m, :], in_=psu[:, :Tmp], func=AF.Identity, bias=b3_sb[:, m : m + 1]
                )

            # ---- pass 2: full stack, per-macro-tile max of L4 preactivation ----
            for t in range(Tm):
                x_sb = xp.tile([3, PT], F16, tag="x", name=f"x2_{t}")
                nc.sync.dma_start(out=x_sb, in_=xTr[:, t * PT : (t + 1) * PT])
                ps1 = ps.tile([128, PT], F32, tag="ps", name=f"ps1b_{t}")
                nc.tensor.matmul(ps1[:, :], w1_sb[:, :], x_sb[:, :], start=True, stop=True)
                h1_sb = h1p.tile([128, PT], F16, tag="h1", name=f"h1b_{t}")
                nc.scalar.activation(out=h1_sb, in_=ps1, func=AF.Relu, bias=b1_sb[:, 0:1])
                f2_sb = f2p.tile([128, 2, PT], F16, tag="f2", name=f"f2_{t}")
                for c in range(2):
                    ps2 = ps.tile([128, PT], F32, tag="ps", name=f"ps2b_{t}_{c}")
                    nc.tensor.matmul(ps2[:, :], w2_sb[:, c, :], h1_sb[:, :], start=True, stop=True)
                    nc.scalar.activation(
                        out=f2_sb[:, c, :], in_=ps2, func=AF.Relu, bias=b2_sb[:, c : c + 1]
                    )
                h3_sb = h3p.tile([128, 4, PT], F16, tag="h3", name=f"h3_{t}")
                for m in range(4):
                    ps3 = ps.tile([128, PT], F32, tag="ps", name=f"ps3_{t}_{m}")
                    nc.tensor.matmul(ps3[:, :], w3a_sb[:, 0, m, :], f2_sb[:, 0, :], start=True, stop=False)
                    nc.tensor.matmul(ps3[:, :], w3a_sb[:, 1, m, :], f2_sb[:, 1, :], start=False, stop=True)
                    nc.scalar.activation(
                        out=h3_sb[:, m, :], in_=ps3, func=AF.Relu, bias=U_sb[:, m, t : t + 1]
                    )
                for m in range(8):
                    ps4 = ps.tile([128, PT], F32, tag="ps", name=f"ps4_{t}_{m}")
                    for k in range(4):
                        nc.tensor.matmul(
                            ps4[:, :], w4_sb[:, k, m, :], h3_sb[:, k, :],
                            start=(k == 0), stop=(k == 3),
                        )
                    nc.vector.tensor_reduce(
                        out=Mx4_sb[:, m, t : t + 1], in_=ps4, axis=AXX, op=ALU_MAX
                    )

            nc.sync.dma_start(out=mx4.ap(), in_=Mx4_sb)

    nc.finalize()
    return nc


def _prepare(x, seg_ids, B):
    """Pad + pack segments into per-core macro-tile streams."""
    counts = np.bincount(seg_ids, minlength=B)
    starts = np.concatenate([[0], np.cumsum(counts)])
    seg_tiles = [(int(c) + PT - 1) // PT for c in counts]

    SLOTS = (B + N_CORES - 1) // N_CORES
    order = np.argsort(-np.asarray(seg_tiles), kind="stable")
    core_segs: list[list[int]] = [[] for _ in range(N_CORES)]
    core_load = [0] * N_CORES
    for s in order:
        cands = [c for c in range(N_CORES) if len(core_segs[c]) < SLOTS]
        c = min(cands, key=lambda i: core_load[i])
        core_segs[c].append(int(s))
        core_load[c] += seg_tiles[s]
    Tm = max(core_load)

    xT_cores, mask_cores, post = [], [], []
    for c in range(N_CORES):
        pts_list, slot_of_tile = [], []
        for slot, s in enumerate(core_segs[c]):
            seg_pts = x[starts[s] : starts[s + 1]]
            ntile = seg_tiles[s]
            padn = ntile * PT - len(seg_pts)
            if padn:
                seg_pts = np.concatenate([seg_pts, seg_pts[:padn]])
            pts_list.append(seg_pts)
            slot_of_tile += [slot] * ntile
        extra = Tm - core_load[c]
        if extra:
            pts_list.append(np.tile(pts_list[0][:PT], (extra, 1)))
            slot_of_tile += [0] * extra
        xc = np.concatenate(pts_list).astype(np.float16)
        xT_cores.append(np.ascontiguousarray(xc.T))
        sot = np.asarray(slot_of_tile)
        Tmp = Tm + (Tm % 2)
        m01 = np.zeros((SLOTS, Tmp), np.float32)
        m01[:, :Tm] = sot[None, :] == np.arange(SLOTS)[:, None]
        mask_cores.append(np.ascontiguousarray(np.broadcast_to(m01[None], (128, SLOTS, Tmp))))
        post.append((core_segs[c], sot))
    return Tm, SLOTS, xT_cores, mask_cores, post


def make_in_maps(inputs):
    """Fold BN, pack points, and build the per-core SPMD input dicts.

    Returns (key, in_maps, post, b4f) where key indexes _PROGRAM_CACHE.
    """
    x = np.asarray(inputs["x"], np.float32)
    seg_ids = np.asarray(inputs["seg_ids"])
    B = int(inputs["num_segments"])

    Wf, bf = [], []
    for i in (1, 2, 3, 4):
        W = np.asarray(inputs[f"W{i}"], np.float32)
        b = np.asarray(inputs[f"b{i}"], np.float32)
        ga = np.asarray(inputs[f"g{i}"], np.float32)
        be = np.asarray(inputs[f"be{i}"], np.float32)
        m = np.asarray(inputs[f"m{i}"], np.float32)
        v = np.asarray(inputs[f"v{i}"], np.float32)
        sc = ga / np.sqrt(v + EPS)
        Wf.append(np.ascontiguousarray(W * sc[None, :]))
        bf.append((b - m) * sc + be)
    W1f, W2f, W3f, W4f = Wf
    b1f, b2f, b3f, b4f = bf

    Tm, SLOTS, xT_cores, mask_cores, post = _prepare(x, seg_ids, B)

    w1d = W1f.astype(np.float16)
    w2d = np.ascontiguousarray(W2f.reshape(128, 2, 128).astype(np.float16))
    w3ad = np.ascontiguousarray(W3f[:256].reshape(2, 128, 4, 128).transpose(1, 0, 2, 3).astype(np.float16))
    w3bd = np.ascontiguousarray(W3f[256:].reshape(2, 128, 4, 128).transpose(1, 0, 2, 3).astype(np.float16))
    w4d = np.ascontiguousarray(W4f.reshape(4, 128, 8, 128).transpose(1, 0, 2, 3).astype(np.float16))
    b1d = np.ascontiguousarray(b1f.reshape(128, 1))
    b2d = np.ascontiguousarray(b2f.reshape(2, 128).T)
    b3d = np.ascontiguousarray(b3f.reshape(4, 128).T)

    in_maps = [
        {
            "xT": xT_cores[c],
            "mask": mask_cores[c],
            "w1": w1d,
            "w2": w2d,
            "w3a": w3ad,
            "w3b": w3bd,
            "w4": w4d,
            "b1": b1d,
            "b2": b2d,
            "b3": b3d,
        }
        for c in range(N_CORES)
    ]
    return (Tm, SLOTS), in_maps, post, b4f


def postprocess(results, post, b4f, B):
    out = np.zeros((B, 1024), np.float32)
    for c in range(N_CORES):
        mx4 = results[c]["mx4"]  # [128, 8, Tm]
        segs, sot = post[c]
        for slot, s in enumerate(segs):
            cols = np.flatnonzero(sot == slot)
            raw = mx4[:, :, cols].max(axis=2)  # [128, 8]
            out[s] = np.maximum(raw.T.reshape(1024) + b4f, 0.0)
    return out


def get_program(key):
    if key not in _PROGRAM_CACHE:
        _PROGRAM_CACHE[key] = _build_program(*key)
    return _PROGRAM_CACHE[key]


def kernel(**inputs) -> np.ndarray:
    B = int(inputs["num_segments"])
    key, in_maps, post, b4f = make_in_maps(inputs)
    nc = get_program(key)
    res = run_bass_kernel_spmd(nc, in_maps, core_ids=list(range(N_CORES)))
    return postprocess(res.results, post, b4f, B)


# revision 6
# speedup vs baseline: 1.1868x; 1.1868x over previous
"""PointNet-style encoder (conv1x1 stack + ragged segment-max) on 8 Trainium2 cores.

Strategy
--------
* BN folded into the conv weights host-side; every layer becomes matmul+bias+ReLU.
* Feature-major on device: activations live as [C, points] tiles, points stream
  through the PE as the matmul free dimension in 512-point macro-tiles.
* Raggedness handled host-side: each segment's points padded to a multiple of
  512 by duplicating its own points (exact under max-pooling), so every
  macro-tile belongs to exactly one segment. Segments are bin-packed 4 per
  core, and every core is padded to the same macro-tile count Tm with
  duplicate tiles, so a single SPMD program covers all cores.
* The mid-network segment-max feeds back via the concat identity
  concat(f2, g) @ W3 = f2 @ W3[:256] + g @ W3[256:]; the g-contribution plus
  b3 becomes a per-macro-tile bias column (table U), applied by the activation
  instruction's per-partition bias operand.
* Per-macro-tile layer-4 maxes are returned raw; the host applies
  relu(. + b4) and the per-segment max over tiles (monotonicity makes this
  exact).
* Matmuls run in float16 (fp32 PSUM accumulate): 1 cycle/column, ~1e-3 rel err.
  (float32r measured 2 cycles/column on HW: it is a 2-pass replicated mode.)
"""

import numpy as np

import concourse.bass as bass
import concourse.mybir as mybir
import concourse.tile as tile
from concourse import bacc
from concourse.bass_utils import run_bass_kernel_spmd

N_CORES = 8
PT = 512  # points per macro-tile
EPS = 1e-3  # keras BatchNormalization default epsilon

F32 = mybir.dt.float32
F16 = mybir.dt.float16
AF = mybir.ActivationFunctionType
AXX = mybir.AxisListType.X
ALU_MAX = mybir.AluOpType.max

_PROGRAM_CACHE: dict = {}


def _build_program(Tm: int, S: int):
    """One SPMD program for all cores: Tm macro-tiles, S segment slots.

    Phase A computes per-macro-tile layer-2 preactivation maxes (DVE/ACT
    bound, light PE). The assembly turns those into the per-tile bias table
    U. Phase B is software-pipelined with a 2-stage skew — per iteration it
    emits L3(t+1), L1(t+2), L4(t), L2(t+2) — so the PE never waits on the
    ACT engine's PSUM->SBUF activations.
    """
    nc = bacc.Bacc("TRN2")
    Tmp = Tm + (Tm % 2)  # fp32r-era evenness; harmless for fp16

    xT = nc.dram_tensor("xT", [3, Tm * PT], F16, kind="ExternalInput")
    mask = nc.dram_tensor("mask", [128, S, Tmp], F32, kind="ExternalInput")
    w1 = nc.dram_tensor("w1", [3, 128], F16, kind="ExternalInput")
    w2 = nc.dram_tensor("w2", [128, 2, 128], F16, kind="ExternalInput")
    w3a = nc.dram_tensor("w3a", [128, 2, 4, 128], F16, kind="ExternalInput")
    w3b = nc.dram_tensor("w3b", [128, 2, 4, 128], F16, kind="ExternalInput")
    w4 = nc.dram_tensor("w4", [128, 4, 8, 128], F16, kind="ExternalInput")
    b1 = nc.dram_tensor("b1", [128, 1], F32, kind="ExternalInput")
    b2 = nc.dram_tensor("b2", [128, 2], F32, kind="ExternalInput")
    b3 = nc.dram_tensor("b3", [128, 4], F32, kind="ExternalInput")
    mx4 = nc.dram_tensor("mx4", [128, 8, Tm], F32, kind="ExternalOutput")

    with tile.TileContext(nc) as tc:
        with (
            tc.tile_pool(name="const", bufs=1) as constp,
            tc.tile_pool(name="xp", bufs=4) as xp,
            tc.tile_pool(name="h1p", bufs=3) as h1p,
            tc.tile_pool(name="f2p", bufs=3) as f2p,
            tc.tile_pool(name="h3p", bufs=3) as h3p,
            tc.tile_pool(name="tmpp", bufs=4) as tmpp,
        ):
            w1_sb = constp.tile([3, 128], F16)
            nc.sync.dma_start(out=w1_sb, in_=w1.ap())
            w2_sb = constp.tile([128, 2, 128], F16)
            nc.sync.dma_start(out=w2_sb, in_=w2.ap())
            w3a_sb = constp.tile([128, 2, 4, 128], F16)
            nc.sync.dma_start(out=w3a_sb, in_=w3a.ap())
            w3b_sb = constp.tile([128, 2, 4, 128], F16)
            nc.sync.dma_start(out=w3b_sb, in_=w3b.ap())
            w4_sb = constp.tile([128, 4, 8, 128], F16)
            nc.sync.dma_start(out=w4_sb, in_=w4.ap())
            b1_sb = constp.tile([128, 1], F32)
            nc.sync.dma_start(out=b1_sb, in_=b1.ap())
            b2_sb = constp.tile([128, 2], F32)
            nc.sync.dma_start(out=b2_sb, in_=b2.ap())
            b3_sb = constp.tile([128, 4], F32)
            nc.sync.dma_start(out=b3_sb, in_=b3.ap())
            mask_sb = constp.tile([128, S, Tmp], F32)
            nc.sync.dma_start(out=mask_sb, in_=mask.ap())

            Mx2_sb = constp.tile([128, 2, Tm], F32)
            Mx2r_sb = constp.tile([128, 2, Tm], F32)
            g_sb = constp.tile([128, 2, S], F32)
            Gacc_sb = constp.tile([128, 2, Tmp], F32)
            G2_sb = constp.tile([128, 2, Tmp], F16)
            U_sb = constp.tile([128, 4, Tmp], F32)
            Mx4_sb = constp.tile([128, 8, Tm], F32)

            xTr = xT.ap()

            # ---- phase A: per-macro-tile max of the layer-2 preactivation ----
            with (
                tc.tile_pool(name="psA1", bufs=2, space="PSUM") as psA1,
                tc.tile_pool(name="psA2", bufs=2, space="PSUM") as psA2,
            ):
                for t in range(Tm):
                    x_sb = xp.tile([3, PT], F16, tag="x", name=f"x1_{t}")
                    nc.sync.dma_start(out=x_sb, in_=xTr[:, t * PT : (t + 1) * PT])
                    ps1 = psA1.tile([128, PT], F32, tag="psa1", name=f"ps1a_{t}")
                    nc.tensor.matmul(ps1[:, :], w1_sb[:, :], x_sb[:, :], start=True, stop=True)
                    h1_sb = h1p.tile([128, PT], F16, tag="h1", name=f"h1a_{t}")
                    nc.scalar.activation(out=h1_sb, in_=ps1, func=AF.Relu, bias=b1_sb[:, 0:1])
                    ps2 = psA2.tile([128, 2, PT], F32, tag="psa2", name=f"ps2a_{t}")
                    for c in range(2):
                        nc.tensor.matmul(ps2[:, c, :], w2_sb[:, c, :], h1_sb[:, :], start=True, stop=True)
                    nc.vector.tensor_reduce(
                        out=Mx2_sb[:, :, t : t + 1], in_=ps2, axis=AXX, op=ALU_MAX
                    )

                # ---- g assembly: per-slot max via host-provided 0/1 masks ----
                for c in range(2):
                    nc.scalar.activation(
                        out=Mx2r_sb[:, c, :], in_=Mx2_sb[:, c, :], func=AF.Relu, bias=b2_sb[:, c : c + 1]
                    )
                for c in range(2):
                    for s in range(S):
                        tmp = tmpp.tile([128, Tm], F32, tag="tmp", name=f"tmpg_{c}_{s}")
                        nc.vector.tensor_mul(tmp, Mx2r_sb[:, c, :], mask_sb[:, s, :Tm])
                        nc.vector.tensor_reduce(
                            out=g_sb[:, c, s : s + 1], in_=tmp, axis=AXX, op=ALU_MAX
                        )
                # expand g back to a per-tile table G2[:, c, t] = g[:, c, slot(t)]
                for c in range(2):
                    nc.vector.tensor_scalar_mul(Gacc_sb[:, c, :], mask_sb[:, 0, :], g_sb[:, c, 0:1])
                    for s in range(1, S):
                        tmp2 = tmpp.tile([128, Tmp], F32, tag="tmp2", name=f"tmpe_{c}_{s}")
                        nc.vector.tensor_scalar_mul(tmp2, mask_sb[:, s, :], g_sb[:, c, s : s + 1])
                        nc.vector.tensor_add(Gacc_sb[:, c, :], Gacc_sb[:, c, :], tmp2)
                    nc.scalar.copy(G2_sb[:, c, :], Gacc_sb[:, c, :])
                # U[:, m, t] = (W3b.T @ G2)[:, t] + b3  — per-tile bias column for L3
                for m in range(4):
                    psu = psA1.tile([128, PT], F32, tag="psa1", name=f"psu_{m}")
                    nc.tensor.matmul(psu[:, :Tmp], w3b_sb[:, 0, m, :], G2_sb[:, 0, :], start=True, stop=False)
                    nc.tensor.matmul(psu[:, :Tmp], w3b_sb[:, 1, m, :], G2_sb[:, 1, :], start=False, stop=True)
                    nc.scalar.activation(
                        out=U_sb[:, m, :], in_=psu[:, :Tmp], func=AF.Identity, bias=b3_sb[:, m : m + 1]
                    )

            # ---- phase B: full stack, software-pipelined with a 2-stage skew ----
            with (
                tc.tile_pool(name="psB12", bufs=2, space="PSUM") as psB12,
                tc.tile_pool(name="psB3", bufs=2, space="PSUM") as psB3,
                tc.tile_pool(name="psB4", bufs=2, space="PSUM") as psB4,
            ):
                f2_tiles = {}
                h3_tiles = {}

                def emit_L1(t):
                    x_sb = xp.tile([3, PT], F16, tag="x", name=f"x2_{t}")
                    nc.sync.dma_start(out=x_sb, in_=xTr[:, t * PT : (t + 1) * PT])
                    ps1 = psB12.tile([128, PT], F32, tag="ps12", name=f"ps1b_{t}")
                    nc.tensor.matmul(ps1[:, :], w1_sb[:, :], x_sb[:, :], start=True, stop=True)
                    h1_sb = h1p.tile([128, PT], F16, tag="h1", name=f"h1b_{t}")
                    nc.scalar.activation(out=h1_sb, in_=ps1, func=AF.Relu, bias=b1_sb[:, 0:1])
                    return h1_sb

                def emit_L2(t, h1_sb):
                    f2_sb = f2p.tile([128, 2, PT], F16, tag="f2", name=f"f2_{t}")
                    for c in range(2):
                        ps2 = psB12.tile([128, PT], F32, tag="ps12", name=f"ps2b_{t}_{c}")
                        nc.tensor.matmul(ps2[:, :], w2_sb[:, c, :], h1_sb[:, :], start=True, stop=True)
                        nc.scalar.activation(
                            out=f2_sb[:, c, :], in_=ps2, func=AF.Relu, bias=b2_sb[:, c : c + 1]
                        )
                    f2_tiles[t] = f2_sb

                def emit_L3(t):
                    f2_sb = f2_tiles.pop(t)
                    h3_sb = h3p.tile([128, 4, PT], F16, tag="h3", name=f"h3_{t}")
                    for m in range(4):
                        ps3 = psB3.tile([128, PT], F32, tag="ps3", name=f"ps3_{t}_{m}")
                        nc.tensor.matmul(ps3[:, :], w3a_sb[:, 0, m, :], f2_sb[:, 0, :], start=True, stop=False)
                        nc.tensor.matmul(ps3[:, :], w3a_sb[:, 1, m, :], f2_sb[:, 1, :], start=False, stop=True)
                        nc.scalar.activation(
                            out=h3_sb[:, m, :], in_=ps3, func=AF.Relu, bias=U_sb[:, m, t : t + 1]
                        )
                    h3_tiles[t] = h3_sb

                def emit_L4(t):
                    h3_sb = h3_tiles.pop(t)
                    for mg in range(4):  # 2 m-chunks per PSUM tile, grouped reduce
                        ps4 = psB4.tile([128, 2, PT], F32, tag="ps4", name=f"ps4_{t}_{mg}")
                        for mi in range(2):
                            m = mg * 2 + mi
                            for k in range(4):
                                nc.tensor.matmul(
                                    ps4[:, mi, :], w4_sb[:, k, m, :], h3_sb[:, k, :],
                                    start=(k == 0), stop=(k == 3),
                                )
                        nc.vector.tensor_reduce(
                            out=Mx4_sb[:, 2 * mg : 2 * mg + 2, t : t + 1], in_=ps4, axis=AXX, op=ALU_MAX
                        )

                # prologue
                h1_0 = emit_L1(0)
                emit_L2(0, h1_0)
                if Tm > 1:
                    h1_1 = emit_L1(1)
                    emit_L2(1, h1_1)
                emit_L3(0)
                # steady state
                for t in range(Tm):
                    if t + 1 < Tm:
                        emit_L3(t + 1)
                    h1_n = emit_L1(t + 2) if t + 2 < Tm else None
                    emit_L4(t)
                    if h1_n is not None:
                        emit_L2(t + 2, h1_n)

            nc.sync.dma_start(out=mx4.ap(), in_=Mx4_sb)

    nc.finalize()
    return nc


def _prepare(x, seg_ids, B):
    """Pad + pack segments into per-core macro-tile streams."""
    counts = np.bincount(seg_ids, minlength=B)
    starts = np.concatenate([[0], np.cumsum(counts)])
    seg_tiles = [(int(c) + PT - 1) // PT for c in counts]

    SLOTS = (B + N_CORES - 1) // N_CORES
    order = np.argsort(-np.asarray(seg_tiles), kind="stable")
    core_segs: list[list[int]] = [[] for _ in range(N_CORES)]
    core_load = [0] * N_CORES
    for s in order:
        cands = [c for c in range(N_CORES) if len(core_segs[c]) < SLOTS]
        c = min(cands, key=lambda i: core_load[i])
        core_segs[c].append(int(s))
        core_load[c] += seg_tiles[s]
    Tm = max(core_load)

    xT_cores, mask_cores, post = [], [], []
    for c in range(N_CORES):
        pts_list, slot_of_tile = [], []
        for slot, s in enumerate(core_segs[c]):
            seg_pts = x[starts[s] : starts[s + 1]]
            ntile = seg_tiles[s]
            padn = ntile * PT - len(seg_pts)
            if padn:
                seg_pts = np.concatenate([seg_pts, seg_pts[:padn]])
            pts_list.append(seg_pts)
            slot_of_tile += [slot] * ntile
        extra = Tm - core_load[c]
        if extra:
            pts_list.append(np.tile(pts_list[0][:PT], (extra, 1)))
            slot_of_tile += [0] * extra
        xc = np.concatenate(pts_list).astype(np.float16)
        xT_cores.append(np.ascontiguousarray(xc.T))
        sot = np.asarray(slot_of_tile)
        Tmp = Tm + (Tm % 2)
        m01 = np.zeros((SLOTS, Tmp), np.float32)
        m01[:, :Tm] = sot[None, :] == np.arange(SLOTS)[:, None]
        mask_cores.append(np.ascontiguousarray(np.broadcast_to(m01[None], (128, SLOTS, Tmp))))
        post.append((core_segs[c], sot))
    return Tm, SLOTS, xT_cores, mask_cores, post


def make_in_maps(inputs):
    """Fold BN, pack points, and build the per-core SPMD input dicts.

    Returns (key, in_maps, post, b4f) where key indexes _PROGRAM_CACHE.
    """
    x = np.asarray(inputs["x"], np.float32)
    seg_ids = np.asarray(inputs["seg_ids"])
    B = int(inputs["num_segments"])

    Wf, bf = [], []
    for i in (1, 2, 3, 4):
        W = np.asarray(inputs[f"W{i}"], np.float32)
        b = np.asarray(inputs[f"b{i}"], np.float32)
        ga = np.asarray(inputs[f"g{i}"], np.float32)
        be = np.asarray(inputs[f"be{i}"], np.float32)
        m = np.asarray(inputs[f"m{i}"], np.float32)
        v = np.asarray(inputs[f"v{i}"], np.float32)
        sc = ga / np.sqrt(v + EPS)
        Wf.append(np.ascontiguousarray(W * sc[None, :]))
        bf.append((b - m) * sc + be)
    W1f, W2f, W3f, W4f = Wf
    b1f, b2f, b3f, b4f = bf

    Tm, SLOTS, xT_cores, mask_cores, post = _prepare(x, seg_ids, B)

    w1d = W1f.astype(np.float16)
    w2d = np.ascontiguousarray(W2f.reshape(128, 2, 128).astype(np.float16))
    w3ad = np.ascontiguousarray(W3f[:256].reshape(2, 128, 4, 128).transpose(1, 0, 2, 3).astype(np.float16))
    w3bd = np.ascontiguousarray(W3f[256:].reshape(2, 128, 4, 128).transpose(1, 0, 2, 3).astype(np.float16))
    w4d = np.ascontiguousarray(W4f.reshape(4, 128, 8, 128).transpose(1, 0, 2, 3).astype(np.float16))
    b1d = np.ascontiguousarray(b1f.reshape(128, 1))
    b2d = np.ascontiguousarray(b2f.reshape(2, 128).T)
    b3d = np.ascontiguousarray(b3f.reshape(4, 128).T)

    in_maps = [
        {
            "xT": xT_cores[c],
            "mask": mask_cores[c],
            "w1": w1d,
            "w2": w2d,
            "w3a": w3ad,
            "w3b": w3bd,
            "w4": w4d,
            "b1": b1d,
            "b2": b2d,
            "b3": b3d,
        }
        for c in range(N_CORES)
    ]
    return (Tm, SLOTS), in_maps, post, b4f


def postprocess(results, post, b4f, B):
    out = np.zeros((B, 1024), np.float32)
    for c in range(N_CORES):
        mx4 = results[c]["mx4"]  # [128, 8, Tm]
        segs, sot = post[c]
        for slot, s in enumerate(segs):
            cols = np.flatnonzero(sot == slot)
            raw = mx4[:, :, cols].max(axis=2)  # [128, 8]
            out[s] = np.maximum(raw.T.reshape(1024) + b4f, 0.0)
    return out


def get_program(key):
    if key not in _PROGRAM_CACHE:
        _PROGRAM_CACHE[key] = _build_program(*key)
    return _PROGRAM_CACHE[key]


def kernel(**inputs) -> np.ndarray:
    B = int(inputs["num_segments"])
    key, in_maps, post, b4f = make_in_maps(inputs)
    nc = get_program(key)
    res = run_bass_kernel_spmd(nc, in_maps, core_ids=list(range(N_CORES)))
    return postprocess(res.results, post, b4f, B)


# revision 9
# speedup vs baseline: 1.1974x; 1.0089x over previous
"""PointNet-style encoder (conv1x1 stack + ragged segment-max) on 8 Trainium2 cores.

Strategy
--------
* BN folded into the conv weights host-side; every layer becomes matmul+bias+ReLU.
* Feature-major on device: activations live as [C, points] tiles, points stream
  through the PE as the matmul free dimension in 512-point macro-tiles.
* Raggedness handled host-side: each segment's points padded to a multiple of
  512 by duplicating its own points (exact under max-pooling), so every
  macro-tile belongs to exactly one segment. Segments are bin-packed 4 per
  core, and every core is padded to the same macro-tile count Tm with
  duplicate tiles, so a single SPMD program covers all cores.
* The mid-network segment-max feeds back via the concat identity
  concat(f2, g) @ W3 = f2 @ W3[:256] + g @ W3[256:]; the g-contribution plus
  b3 becomes a per-macro-tile bias column (table U), applied by the activation
  instruction's per-partition bias operand.
* Per-macro-tile layer-4 maxes are returned raw; the host applies
  relu(. + b4) and the per-segment max over tiles (monotonicity makes this
  exact).
* Matmuls run in float16 (fp32 PSUM accumulate): 1 cycle/column, ~1e-3 rel err.
  (float32r measured 2 cycles/column on HW: it is a 2-pass replicated mode.)
"""

import numpy as np

import concourse.bass as bass
import concourse.mybir as mybir
import concourse.tile as tile
from concourse import bacc
from concourse.bass_utils import run_bass_kernel_spmd

N_CORES = 8
PT = 512  # points per macro-tile
EPS = 1e-3  # keras BatchNormalization default epsilon

F32 = mybir.dt.float32
F16 = mybir.dt.float16
AF = mybir.ActivationFunctionType
AXX = mybir.AxisListType.X
ALU_MAX = mybir.AluOpType.max

_PROGRAM_CACHE: dict = {}


def _build_program(Tm: int, S: int):
    """One SPMD program for all cores: Tm macro-tiles, S segment slots.

    Phase A computes per-macro-tile layer-2 preactivation maxes (DVE/ACT
    bound, light PE). The assembly turns those into the per-tile bias table
    U. Phase B is software-pipelined with a 2-stage skew — per iteration it
    emits L3(t+1), L1(t+2), L4(t), L2(t+2) — so the PE never waits on the
    ACT engine's PSUM->SBUF activations.
    """
    nc = bacc.Bacc("TRN2")
    Tmp = Tm + (Tm % 2)  # fp32r-era evenness; harmless for fp16

    xT = nc.dram_tensor("xT", [3, Tm * PT], F16, kind="ExternalInput")
    mask = nc.dram_tensor("mask", [128, S, Tmp], F32, kind="ExternalInput")
    w1 = nc.dram_tensor("w1", [3, 128], F16, kind="ExternalInput")
    w2 = nc.dram_tensor("w2", [128, 2, 128], F16, kind="ExternalInput")
    w3a = nc.dram_tensor("w3a", [128, 2, 4, 128], F16, kind="ExternalInput")
    w3b = nc.dram_tensor("w3b", [128, 2, 4, 128], F16, kind="ExternalInput")
    w4 = nc.dram_tensor("w4", [128, 4, 8, 128], F16, kind="ExternalInput")
    b1 = nc.dram_tensor("b1", [128, 1], F32, kind="ExternalInput")
    b2 = nc.dram_tensor("b2", [128, 2], F32, kind="ExternalInput")
    b3 = nc.dram_tensor("b3", [128, 4], F32, kind="ExternalInput")
    mx4 = nc.dram_tensor("mx4", [128, 8, Tm], F32, kind="ExternalOutput")

    with tile.TileContext(nc) as tc:
        with (
            tc.tile_pool(name="const", bufs=1) as constp,
            tc.tile_pool(name="xp", bufs=4) as xp,
            tc.tile_pool(name="h1p", bufs=3) as h1p,
            tc.tile_pool(name="f2p", bufs=3) as f2p,
            tc.tile_pool(name="h3p", bufs=3) as h3p,
            tc.tile_pool(name="tmpp", bufs=4) as tmpp,
        ):
            # small, immediately-needed constants on the sync DMA queue;
            # the big phase-B weights go on the gpsimd queue so they don't
            # head-of-line-block phase A's x-tile loads.
            w1_sb = constp.tile([3, 128], F16)
            nc.sync.dma_start(out=w1_sb, in_=w1.ap())
            w2_sb = constp.tile([128, 2, 128], F16)
            nc.sync.dma_start(out=w2_sb, in_=w2.ap())
            b1_sb = constp.tile([128, 1], F32)
            nc.sync.dma_start(out=b1_sb, in_=b1.ap())
            b2_sb = constp.tile([128, 2], F32)
            nc.sync.dma_start(out=b2_sb, in_=b2.ap())
            b3_sb = constp.tile([128, 4], F32)
            nc.sync.dma_start(out=b3_sb, in_=b3.ap())
            mask_sb = constp.tile([128, S, Tmp], F32)
            nc.sync.dma_start(out=mask_sb, in_=mask.ap())
            w3a_sb = constp.tile([128, 2, 4, 128], F16)
            nc.gpsimd.dma_start(out=w3a_sb, in_=w3a.ap())
            w3b_sb = constp.tile([128, 2, 4, 128], F16)
            nc.gpsimd.dma_start(out=w3b_sb, in_=w3b.ap())
            w4_sb = constp.tile([128, 4, 8, 128], F16)
            nc.gpsimd.dma_start(out=w4_sb, in_=w4.ap())

            Mx2_sb = constp.tile([128, 2, Tm], F32)
            Mx2r_sb = constp.tile([128, 2, Tm], F32)
            g_sb = constp.tile([128, 2, S], F32)
            Gacc_sb = constp.tile([128, 2, Tmp], F32)
            G2_sb = constp.tile([128, 2, Tmp], F16)
            U_sb = constp.tile([128, 4, Tmp], F32)
            Mx4_sb = constp.tile([128, 8, Tm], F32)

            xTr = xT.ap()

            prologue_f2 = {}

            # ---- phase A: per-macro-tile max of the layer-2 preactivation ----
            with (
                tc.tile_pool(name="psA1", bufs=2, space="PSUM") as psA1,
                tc.tile_pool(name="psA2", bufs=3, space="PSUM") as psA2,
            ):
                for t in range(Tm):
                    x_sb = xp.tile([3, PT], F16, tag="x", name=f"x1_{t}")
                    nc.sync.dma_start(out=x_sb, in_=xTr[:, t * PT : (t + 1) * PT])
                    ps1 = psA1.tile([128, PT], F32, tag="psa1", name=f"ps1a_{t}")
                    nc.tensor.matmul(ps1[:, :], w1_sb[:, :], x_sb[:, :], start=True, stop=True)
                    h1_sb = h1p.tile([128, PT], F16, tag="h1", name=f"h1a_{t}")
                    nc.scalar.activation(out=h1_sb, in_=ps1, func=AF.Relu, bias=b1_sb[:, 0:1])
                    ps2 = psA2.tile([128, 2, PT], F32, tag="psa2", name=f"ps2a_{t}")
                    for c in range(2):
                        nc.tensor.matmul(ps2[:, c, :], w2_sb[:, c, :], h1_sb[:, :], start=True, stop=True)
                    nc.vector.tensor_reduce(
                        out=Mx2_sb[:, :, t : t + 1], in_=ps2, axis=AXX, op=ALU_MAX
                    )

                # ---- phase-B prologue (no U dependency): overlap with assembly ----
                for t0 in range(min(2, Tm)):
                    x_sb = xp.tile([3, PT], F16, tag="x", name=f"x2_{t0}")
                    nc.sync.dma_start(out=x_sb, in_=xTr[:, t0 * PT : (t0 + 1) * PT])
                    ps1 = psA1.tile([128, PT], F32, tag="psa1", name=f"ps1b_{t0}")
                    nc.tensor.matmul(ps1[:, :], w1_sb[:, :], x_sb[:, :], start=True, stop=True)
                    h1_sb = h1p.tile([128, PT], F16, tag="h1", name=f"h1b_{t0}")
                    nc.scalar.activation(out=h1_sb, in_=ps1, func=AF.Relu, bias=b1_sb[:, 0:1])
                    f2_sb = f2p.tile([128, 2, PT], F16, tag="f2", name=f"f2_{t0}")
                    for c in range(2):
                        ps2 = psA1.tile([128, PT], F32, tag="psa1", name=f"ps2b_{t0}_{c}")
                        nc.tensor.matmul(ps2[:, :], w2_sb[:, c, :], h1_sb[:, :], start=True, stop=True)
                        nc.scalar.activation(
                            out=f2_sb[:, c, :], in_=ps2, func=AF.Relu, bias=b2_sb[:, c : c + 1]
                        )
                    prologue_f2[t0] = f2_sb

                # ---- g assembly: per-slot max via host-provided 0/1 masks ----
                for c in range(2):
                    nc.scalar.activation(
                        out=Mx2r_sb[:, c, :], in_=Mx2_sb[:, c, :], func=AF.Relu, bias=b2_sb[:, c : c + 1]
                    )
                for c in range(2):
                    for s in range(S):
                        tmp = tmpp.tile([128, Tm], F32, tag="tmp", name=f"tmpg_{c}_{s}")
                        nc.vector.tensor_mul(tmp, Mx2r_sb[:, c, :], mask_sb[:, s, :Tm])
                        nc.vector.tensor_reduce(
                            out=g_sb[:, c, s : s + 1], in_=tmp, axis=AXX, op=ALU_MAX
                        )
                # expand g back to a per-tile table G2[:, c, t] = g[:, c, slot(t)]
                for c in range(2):
                    nc.vector.tensor_scalar_mul(Gacc_sb[:, c, :], mask_sb[:, 0, :], g_sb[:, c, 0:1])
                    for s in range(1, S):
                        tmp2 = tmpp.tile([128, Tmp], F32, tag="tmp2", name=f"tmpe_{c}_{s}")
                        nc.vector.tensor_scalar_mul(tmp2, mask_sb[:, s, :], g_sb[:, c, s : s + 1])
                        nc.vector.tensor_add(Gacc_sb[:, c, :], Gacc_sb[:, c, :], tmp2)
                    nc.scalar.copy(G2_sb[:, c, :], Gacc_sb[:, c, :])
                # U[:, m, t] = (W3b.T @ G2)[:, t] + b3  — per-tile bias column for L3
                for m in range(4):
                    psu = psA1.tile([128, PT], F32, tag="psa1", name=f"psu_{m}")
                    nc.tensor.matmul(psu[:, :Tmp], w3b_sb[:, 0, m, :], G2_sb[:, 0, :], start=True, stop=False)
                    nc.tensor.matmul(psu[:, :Tmp], w3b_sb[:, 1, m, :], G2_sb[:, 1, :], start=False, stop=True)
                    nc.scalar.activation(
                        out=U_sb[:, m, :], in_=psu[:, :Tmp], func=AF.Identity, bias=b3_sb[:, m : m + 1]
                    )

            # ---- phase B: full stack, software-pipelined with a 2-stage skew ----
            with (
                tc.tile_pool(name="psB12", bufs=2, space="PSUM") as psB12,
                tc.tile_pool(name="psB3", bufs=2, space="PSUM") as psB3,
                tc.tile_pool(name="psB4", bufs=2, space="PSUM") as psB4,
            ):
                f2_tiles = dict(prologue_f2)
                h3_tiles = {}

                def emit_L1(t):
                    x_sb = xp.tile([3, PT], F16, tag="x", name=f"x2_{t}")
                    nc.sync.dma_start(out=x_sb, in_=xTr[:, t * PT : (t + 1) * PT])
                    ps1 = psB12.tile([128, PT], F32, tag="ps12", name=f"ps1b_{t}")
                    nc.tensor.matmul(ps1[:, :], w1_sb[:, :], x_sb[:, :], start=True, stop=True)
                    h1_sb = h1p.tile([128, PT], F16, tag="h1", name=f"h1b_{t}")
                    nc.scalar.activation(out=h1_sb, in_=ps1, func=AF.Relu, bias=b1_sb[:, 0:1])
                    return h1_sb

                def emit_L2(t, h1_sb):
                    f2_sb = f2p.tile([128, 2, PT], F16, tag="f2", name=f"f2_{t}")
                    for c in range(2):
                        ps2 = psB12.tile([128, PT], F32, tag="ps12", name=f"ps2b_{t}_{c}")
                        nc.tensor.matmul(ps2[:, :], w2_sb[:, c, :], h1_sb[:, :], start=True, stop=True)
                        nc.scalar.activation(
                            out=f2_sb[:, c, :], in_=ps2, func=AF.Relu, bias=b2_sb[:, c : c + 1]
                        )
                    f2_tiles[t] = f2_sb

                def emit_L3(t):
                    f2_sb = f2_tiles.pop(t)
                    h3_sb = h3p.tile([128, 4, PT], F16, tag="h3", name=f"h3_{t}")
                    for m in range(4):
                        ps3 = psB3.tile([128, PT], F32, tag="ps3", name=f"ps3_{t}_{m}")
                        nc.tensor.matmul(ps3[:, :], w3a_sb[:, 0, m, :], f2_sb[:, 0, :], start=True, stop=False)
                        nc.tensor.matmul(ps3[:, :], w3a_sb[:, 1, m, :], f2_sb[:, 1, :], start=False, stop=True)
                        nc.scalar.activation(
                            out=h3_sb[:, m, :], in_=ps3, func=AF.Relu, bias=U_sb[:, m, t : t + 1]
                        )
                    h3_tiles[t] = h3_sb

                def emit_L4(t):
                    h3_sb = h3_tiles.pop(t)
                    for mg in range(4):  # 2 m-chunks per PSUM tile, grouped reduce
                        ps4 = psB4.tile([128, 2, PT], F32, tag="ps4", name=f"ps4_{t}_{mg}")
                        for mi in range(2):
                            m = mg * 2 + mi
                            for k in range(4):
                                nc.tensor.matmul(
                                    ps4[:, mi, :], w4_sb[:, k, m, :], h3_sb[:, k, :],
                                    start=(k == 0), stop=(k == 3),
                                )
                        nc.vector.tensor_reduce(
                            out=Mx4_sb[:, 2 * mg : 2 * mg + 2, t : t + 1], in_=ps4, axis=AXX, op=ALU_MAX
                        )

                # prologue (tiles 0/1 were produced during assembly)
                emit_L3(0)
                # steady state
                for t in range(Tm):
                    if t + 1 < Tm:
                        emit_L3(t + 1)
                    h1_n = emit_L1(t + 2) if t + 2 < Tm else None
                    emit_L4(t)
                    if h1_n is not None:
                        emit_L2(t + 2, h1_n)

            nc.sync.dma_start(out=mx4.ap(), in_=Mx4_sb)

    nc.finalize()
    return nc


def _prepare(x, seg_ids, B):
    """Pad + pack segments into per-core macro-tile streams."""
    counts = np.bincount(seg_ids, minlength=B)
    starts = np.concatenate([[0], np.cumsum(counts)])
    seg_tiles = [(int(c) + PT - 1) // PT for c in counts]

    SLOTS = (B + N_CORES - 1) // N_CORES
    order = np.argsort(-np.asarray(seg_tiles), kind="stable")
    core_segs: list[list[int]] = [[] for _ in range(N_CORES)]
    core_load = [0] * N_CORES
    for s in order:
        cands = [c for c in range(N_CORES) if len(core_segs[c]) < SLOTS]
        c = min(cands, key=lambda i: core_load[i])
        core_segs[c].append(int(s))
        core_load[c] += seg_tiles[s]

    # local search: swap segments between cores to shave the max load
    ideal = (sum(seg_tiles) + N_CORES - 1) // N_CORES
    for _ in range(200):
        if max(core_load) <= ideal:
            break
        hi = max(range(N_CORES), key=lambda i: core_load[i])
        improved = False
        for lo in sorted(range(N_CORES), key=lambda i: core_load[i]):
            if lo == hi:
                continue
            for ia, sa in enumerate(core_segs[hi]):
                for ib, sb in enumerate(core_segs[lo]):
                    d = seg_tiles[sa] - seg_tiles[sb]
                    if d > 0 and max(core_load[hi] - d, core_load[lo] + d) < max(
                        core_load[hi], core_load[lo]
                    ):
                        core_segs[hi][ia], core_segs[lo][ib] = sb, sa
                        core_load[hi] -= d
                        core_load[lo] += d
                        improved = True
                        break
                if improved:
                    break
            if improved:
                break
        if not improved:
            break
    Tm = max(core_load)

    xT_cores, mask_cores, post = [], [], []
    for c in range(N_CORES):
        pts_list, slot_of_tile = [], []
        for slot, s in enumerate(core_segs[c]):
            seg_pts = x[starts[s] : starts[s + 1]]
            ntile = seg_tiles[s]
            padn = ntile * PT - len(seg_pts)
            if padn:
                seg_pts = np.concatenate([seg_pts, seg_pts[:padn]])
            pts_list.append(seg_pts)
            slot_of_tile += [slot] * ntile
        extra = Tm - core_load[c]
        if extra:
            pts_list.append(np.tile(pts_list[0][:PT], (extra, 1)))
            slot_of_tile += [0] * extra
        xc = np.concatenate(pts_list).astype(np.float16)
        xT_cores.append(np.ascontiguousarray(xc.T))
        sot = np.asarray(slot_of_tile)
        Tmp = Tm + (Tm % 2)
        m01 = np.zeros((SLOTS, Tmp), np.float32)
        m01[:, :Tm] = sot[None, :] == np.arange(SLOTS)[:, None]
        mask_cores.append(np.ascontiguousarray(np.broadcast_to(m01[None], (128, SLOTS, Tmp))))
        post.append((core_segs[c], sot))
    return Tm, SLOTS, xT_cores, mask_cores, post


def make_in_maps(inputs):
    """Fold BN, pack points, and build the per-core SPMD input dicts.

    Returns (key, in_maps, post, b4f) where key indexes _PROGRAM_CACHE.
    """
    x = np.asarray(inputs["x"], np.float32)
    seg_ids = np.asarray(inputs["seg_ids"])
    B = int(inputs["num_segments"])

    Wf, bf = [], []
    for i in (1, 2, 3, 4):
        W = np.asarray(inputs[f"W{i}"], np.float32)
        b = np.asarray(inputs[f"b{i}"], np.float32)
        ga = np.asarray(inputs[f"g{i}"], np.float32)
        be = np.asarray(inputs[f"be{i}"], np.float32)
        m = np.asarray(inputs[f"m{i}"], np.float32)
        v = np.asarray(inputs[f"v{i}"], np.float32)
        sc = ga / np.sqrt(v + EPS)
        Wf.append(np.ascontiguousarray(W * sc[None, :]))
        bf.append((b - m) * sc + be)
    W1f, W2f, W3f, W4f = Wf
    b1f, b2f, b3f, b4f = bf

    Tm, SLOTS, xT_cores, mask_cores, post = _prepare(x, seg_ids, B)

    w1d = W1f.astype(np.float16)
    w2d = np.ascontiguousarray(W2f.reshape(128, 2, 128).astype(np.float16))
    w3ad = np.ascontiguousarray(W3f[:256].reshape(2, 128, 4, 128).transpose(1, 0, 2, 3).astype(np.float16))
    w3bd = np.ascontiguousarray(W3f[256:].reshape(2, 128, 4, 128).transpose(1, 0, 2, 3).astype(np.float16))
    w4d = np.ascontiguousarray(W4f.reshape(4, 128, 8, 128).transpose(1, 0, 2, 3).astype(np.float16))
    b1d = np.ascontiguousarray(b1f.reshape(128, 1))
    b2d = np.ascontiguousarray(b2f.reshape(2, 128).T)
    b3d = np.ascontiguousarray(b3f.reshape(4, 128).T)

    in_maps = [
        {
            "xT": xT_cores[c],
            "mask": mask_cores[c],
            "w1": w1d,
            "w2": w2d,
            "w3a": w3ad,
            "w3b": w3bd,
            "w4": w4d,
            "b1": b1d,
            "b2": b2d,
            "b3": b3d,
        }
        for c in range(N_CORES)
    ]
    return (Tm, SLOTS), in_maps, post, b4f


def postprocess(results, post, b4f, B):
    out = np.zeros((B, 1024), np.float32)
    for c in range(N_CORES):
        mx4 = results[c]["mx4"]  # [128, 8, Tm]
        segs, sot = post[c]
        for slot, s in enumerate(segs):
            cols = np.flatnonzero(sot == slot)
            raw = mx4[:, :, cols].max(axis=2)  # [128, 8]
            out[s] = np.maximum(raw.T.reshape(1024) + b4f, 0.0)
    return out


def get_program(key):
    if key not in _PROGRAM_CACHE:
        _PROGRAM_CACHE[key] = _build_program(*key)
    return _PROGRAM_CACHE[key]


def kernel(**inputs) -> np.ndarray:
    B = int(inputs["num_segments"])
    key, in_maps, post, b4f = make_in_maps(inputs)
    nc = get_program(key)
    res = run_bass_kernel_spmd(nc, in_maps, core_ids=list(range(N_CORES)))
    return postprocess(res.results, post, b4f, B)


# revision 10
# speedup vs baseline: 1.2339x; 1.0305x over previous
"""PointNet-style encoder (conv1x1 stack + ragged segment-max) on 8 Trainium2 cores.

Strategy
--------
* BN folded into the conv weights host-side; every layer becomes matmul+bias+ReLU.
* Feature-major on device: activations live as [C, points] tiles, points stream
  through the PE as the matmul free dimension in 512-point macro-tiles.
* Raggedness handled host-side: each segment's points padded to a multiple of
  512 by duplicating its own points (exact under max-pooling), so every
  macro-tile belongs to exactly one segment. Segments are bin-packed 4 per
  core, and every core is padded to the same macro-tile count Tm with
  duplicate tiles, so a single SPMD program covers all cores.
* The mid-network segment-max feeds back via the concat identity
  concat(f2, g) @ W3 = f2 @ W3[:256] + g @ W3[256:]; the g-contribution plus
  b3 becomes a per-macro-tile bias column (table U), applied by the activation
  instruction's per-partition bias operand.
* Per-macro-tile layer-4 maxes are returned raw; the host applies
  relu(. + b4) and the per-segment max over tiles (monotonicity makes this
  exact).
* Matmuls run in float16 (fp32 PSUM accumulate): 1 cycle/column, ~1e-3 rel err.
  (float32r measured 2 cycles/column on HW: it is a 2-pass replicated mode.)
"""

import numpy as np

import concourse.bass as bass
import concourse.mybir as mybir
import concourse.tile as tile
from concourse import bacc
from concourse.bass_utils import run_bass_kernel_spmd

N_CORES = 8
PT = 512  # points per macro-tile
EPS = 1e-3  # keras BatchNormalization default epsilon

F32 = mybir.dt.float32
F16 = mybir.dt.float16
AF = mybir.ActivationFunctionType
AXX = mybir.AxisListType.X
ALU_MAX = mybir.AluOpType.max

_PROGRAM_CACHE: dict = {}


def _build_program(Tm: int, S: int):
    """One SPMD program for all cores: Tm macro-tiles, S segment slots.

    Phase A computes per-macro-tile layer-2 preactivation maxes (DVE/ACT
    bound, light PE). The assembly turns those into the per-tile bias table
    U. Phase B is software-pipelined with a 2-stage skew — per iteration it
    emits L3(t+1), L1(t+2), L4(t), L2(t+2) — so the PE never waits on the
    ACT engine's PSUM->SBUF activations.
    """
    nc = bacc.Bacc("TRN2")
    Tmp = Tm + (Tm % 2)  # fp32r-era evenness; harmless for fp16

    xT = nc.dram_tensor("xT", [3, Tm * PT], F16, kind="ExternalInput")
    mask = nc.dram_tensor("mask", [128, S, Tmp], F32, kind="ExternalInput")
    w1 = nc.dram_tensor("w1", [3, 128], F16, kind="ExternalInput")
    w2 = nc.dram_tensor("w2", [128, 2, 128], F16, kind="ExternalInput")
    w3a = nc.dram_tensor("w3a", [128, 2, 4, 128], F16, kind="ExternalInput")
    w3b = nc.dram_tensor("w3b", [128, 2, 4, 128], F16, kind="ExternalInput")
    w4 = nc.dram_tensor("w4", [128, 4, 8, 128], F16, kind="ExternalInput")
    b1 = nc.dram_tensor("b1", [128, 1], F32, kind="ExternalInput")
    b2 = nc.dram_tensor("b2", [128, 2], F32, kind="ExternalInput")
    b3 = nc.dram_tensor("b3", [128, 4], F32, kind="ExternalInput")
    mx4 = nc.dram_tensor("mx4", [128, 8, Tm], F32, kind="ExternalOutput")

    with tile.TileContext(nc) as tc:
        with (
            tc.tile_pool(name="const", bufs=1) as constp,
            tc.tile_pool(name="xp", bufs=4) as xp,
            tc.tile_pool(name="h1p", bufs=3) as h1p,
            tc.tile_pool(name="f2p", bufs=3) as f2p,
            tc.tile_pool(name="h3p", bufs=3) as h3p,
            tc.tile_pool(name="tmpp", bufs=4) as tmpp,
        ):
            # small, immediately-needed constants on the sync DMA queue;
            # the big phase-B weights go on the gpsimd queue so they don't
            # head-of-line-block phase A's x-tile loads.
            w1_sb = constp.tile([3, 128], F16)
            nc.sync.dma_start(out=w1_sb, in_=w1.ap())
            w2_sb = constp.tile([128, 2, 128], F16)
            nc.sync.dma_start(out=w2_sb, in_=w2.ap())
            b1_sb = constp.tile([128, 1], F32)
            nc.sync.dma_start(out=b1_sb, in_=b1.ap())
            b2_sb = constp.tile([128, 2], F32)
            nc.sync.dma_start(out=b2_sb, in_=b2.ap())
            b3_sb = constp.tile([128, 4], F32)
            nc.sync.dma_start(out=b3_sb, in_=b3.ap())
            mask_sb = constp.tile([128, S, Tmp], F32)
            nc.sync.dma_start(out=mask_sb, in_=mask.ap())
            w3a_sb = constp.tile([128, 2, 4, 128], F16)
            nc.gpsimd.dma_start(out=w3a_sb, in_=w3a.ap())
            w3b_sb = constp.tile([128, 2, 4, 128], F16)
            nc.gpsimd.dma_start(out=w3b_sb, in_=w3b.ap())
            w4_sb = constp.tile([128, 4, 8, 128], F16)
            nc.gpsimd.dma_start(out=w4_sb, in_=w4.ap())

            Mx2_sb = constp.tile([128, 2, Tm], F32)
            Mx2r_sb = constp.tile([128, 2, Tm], F32)
            g_sb = constp.tile([128, 2, S], F32)
            Gacc_sb = constp.tile([128, 2, Tmp], F32)
            G2_sb = constp.tile([128, 2, Tmp], F16)
            U_sb = constp.tile([128, 4, Tmp], F32)
            Mx4_sb = constp.tile([128, 8, Tm], F32)

            xTr = xT.ap()

            prologue_f2 = {}

            # ---- phase A: per-macro-tile max of the layer-2 preactivation ----
            with (
                tc.tile_pool(name="psA1", bufs=2, space="PSUM") as psA1,
                tc.tile_pool(name="psA2", bufs=3, space="PSUM") as psA2,
            ):
                # HAM warmup: ~48 dependency-free matmuls on scratch data keep
                # the PE busy through the DMA prologue so the clock gate opens
                # (1.2 -> 2.4 GHz) before phase A's real matmuls start.
                warm_src = constp.tile([128, PT], F16, name="warm_src")
                nc.vector.memset(warm_src, 0.01)
                warm_out = constp.tile([128, 1], F32, name="warm_out")
                ps_w = psA1.tile([128, PT], F32, tag="psa1", name="ps_warm")
                for i in range(48):
                    nc.tensor.matmul(
                        ps_w[:, :], warm_src[:, 0:128], warm_src[:, :],
                        start=True, stop=True,
                    )
                nc.vector.tensor_reduce(
                    out=warm_out, in_=ps_w[:, 0:8], axis=AXX, op=ALU_MAX
                )

                # skewed: emit L1(t+1) before L2(t) so the PE never waits on
                # the h1 activation.
                h1s = {}

                def emit_A1(t):
                    x_sb = xp.tile([3, PT], F16, tag="x", name=f"x1_{t}")
                    nc.sync.dma_start(out=x_sb, in_=xTr[:, t * PT : (t + 1) * PT])
                    ps1 = psA1.tile([128, PT], F32, tag="psa1", name=f"ps1a_{t}")
                    nc.tensor.matmul(ps1[:, :], w1_sb[:, :], x_sb[:, :], start=True, stop=True)
                    h1_sb = h1p.tile([128, PT], F16, tag="h1", name=f"h1a_{t}")
                    nc.scalar.activation(out=h1_sb, in_=ps1, func=AF.Relu, bias=b1_sb[:, 0:1])
                    h1s[t] = h1_sb

                emit_A1(0)
                for t in range(Tm):
                    if t + 1 < Tm:
                        emit_A1(t + 1)
                    h1_sb = h1s.pop(t)
                    ps2 = psA2.tile([128, 2, PT], F32, tag="psa2", name=f"ps2a_{t}")
                    for c in range(2):
                        nc.tensor.matmul(ps2[:, c, :], w2_sb[:, c, :], h1_sb[:, :], start=True, stop=True)
                    nc.vector.tensor_reduce(
                        out=Mx2_sb[:, :, t : t + 1], in_=ps2, axis=AXX, op=ALU_MAX
                    )

                # ---- phase-B prologue (no U dependency): overlap with assembly ----
                for t0 in range(min(2, Tm)):
                    x_sb = xp.tile([3, PT], F16, tag="x", name=f"x2_{t0}")
                    nc.sync.dma_start(out=x_sb, in_=xTr[:, t0 * PT : (t0 + 1) * PT])
                    ps1 = psA1.tile([128, PT], F32, tag="psa1", name=f"ps1b_{t0}")
                    nc.tensor.matmul(ps1[:, :], w1_sb[:, :], x_sb[:, :], start=True, stop=True)
                    h1_sb = h1p.tile([128, PT], F16, tag="h1", name=f"h1b_{t0}")
                    nc.scalar.activation(out=h1_sb, in_=ps1, func=AF.Relu, bias=b1_sb[:, 0:1])
                    f2_sb = f2p.tile([128, 2, PT], F16, tag="f2", name=f"f2_{t0}")
                    for c in range(2):
                        ps2 = psA1.tile([128, PT], F32, tag="psa1", name=f"ps2b_{t0}_{c}")
                        nc.tensor.matmul(ps2[:, :], w2_sb[:, c, :], h1_sb[:, :], start=True, stop=True)
                        nc.scalar.activation(
                            out=f2_sb[:, c, :], in_=ps2, func=AF.Relu, bias=b2_sb[:, c : c + 1]
                        )
                    prologue_f2[t0] = f2_sb

                # ---- g assembly: per-slot max via host-provided 0/1 masks ----
                for c in range(2):
                    nc.scalar.activation(
                        out=Mx2r_sb[:, c, :], in_=Mx2_sb[:, c, :], func=AF.Relu, bias=b2_sb[:, c : c + 1]
                    )
                for c in range(2):
                    for s in range(S):
                        tmp = tmpp.tile([128, Tm], F32, tag="tmp", name=f"tmpg_{c}_{s}")
                        nc.vector.tensor_mul(tmp, Mx2r_sb[:, c, :], mask_sb[:, s, :Tm])
                        nc.vector.tensor_reduce(
                            out=g_sb[:, c, s : s + 1], in_=tmp, axis=AXX, op=ALU_MAX
                        )
                # expand g back to a per-tile table G2[:, c, t] = g[:, c, slot(t)]
                for c in range(2):
                    nc.vector.tensor_scalar_mul(Gacc_sb[:, c, :], mask_sb[:, 0, :], g_sb[:, c, 0:1])
                    for s in range(1, S):
                        tmp2 = tmpp.tile([128, Tmp], F32, tag="tmp2", name=f"tmpe_{c}_{s}")
                        nc.vector.tensor_scalar_mul(tmp2, mask_sb[:, s, :], g_sb[:, c, s : s + 1])
                        nc.vector.tensor_add(Gacc_sb[:, c, :], Gacc_sb[:, c, :], tmp2)
                    nc.scalar.copy(G2_sb[:, c, :], Gacc_sb[:, c, :])
                # U[:, m, t] = (W3b.T @ G2)[:, t] + b3  — per-tile bias column for L3
                for m in range(4):
                    psu = psA1.tile([128, PT], F32, tag="psa1", name=f"psu_{m}")
                    nc.tensor.matmul(psu[:, :Tmp], w3b_sb[:, 0, m, :], G2_sb[:, 0, :], start=True, stop=False)
                    nc.tensor.matmul(psu[:, :Tmp], w3b_sb[:, 1, m, :], G2_sb[:, 1, :], start=False, stop=True)
                    nc.scalar.activation(
                        out=U_sb[:, m, :], in_=psu[:, :Tmp], func=AF.Identity, bias=b3_sb[:, m : m + 1]
                    )

            # ---- phase B: full stack, software-pipelined with a 2-stage skew ----
            with (
                tc.tile_pool(name="psB12", bufs=2, space="PSUM") as psB12,
                tc.tile_pool(name="psB3", bufs=2, space="PSUM") as psB3,
                tc.tile_pool(name="psB4", bufs=2, space="PSUM") as psB4,
            ):
                f2_tiles = dict(prologue_f2)
                h3_tiles = {}

                def emit_L1(t):
                    x_sb = xp.tile([3, PT], F16, tag="x", name=f"x2_{t}")
                    nc.sync.dma_start(out=x_sb, in_=xTr[:, t * PT : (t + 1) * PT])
                    ps1 = psB12.tile([128, PT], F32, tag="ps12", name=f"ps1b_{t}")
                    nc.tensor.matmul(ps1[:, :], w1_sb[:, :], x_sb[:, :], start=True, stop=True)
                    h1_sb = h1p.tile([128, PT], F16, tag="h1", name=f"h1b_{t}")
                    nc.scalar.activation(out=h1_sb, in_=ps1, func=AF.Relu, bias=b1_sb[:, 0:1])
                    return h1_sb

                def emit_L2(t, h1_sb):
                    f2_sb = f2p.tile([128, 2, PT], F16, tag="f2", name=f"f2_{t}")
                    for c in range(2):
                        ps2 = psB12.tile([128, PT], F32, tag="ps12", name=f"ps2b_{t}_{c}")
                        nc.tensor.matmul(ps2[:, :], w2_sb[:, c, :], h1_sb[:, :], start=True, stop=True)
                        nc.scalar.activation(
                            out=f2_sb[:, c, :], in_=ps2, func=AF.Relu, bias=b2_sb[:, c : c + 1]
                        )
                    f2_tiles[t] = f2_sb

                def emit_L3(t):
                    f2_sb = f2_tiles.pop(t)
                    h3_sb = h3p.tile([128, 4, PT], F16, tag="h3", name=f"h3_{t}")
                    for m in range(4):
                        ps3 = psB3.tile([128, PT], F32, tag="ps3", name=f"ps3_{t}_{m}")
                        nc.tensor.matmul(ps3[:, :], w3a_sb[:, 0, m, :], f2_sb[:, 0, :], start=True, stop=False)
                        nc.tensor.matmul(ps3[:, :], w3a_sb[:, 1, m, :], f2_sb[:, 1, :], start=False, stop=True)
                        nc.scalar.activation(
                            out=h3_sb[:, m, :], in_=ps3, func=AF.Relu, bias=U_sb[:, m, t : t + 1]
                        )
                    h3_tiles[t] = h3_sb

                def emit_L4(t):
                    h3_sb = h3_tiles.pop(t)
                    for mg in range(4):  # 2 m-chunks per PSUM tile, grouped reduce
                        ps4 = psB4.tile([128, 2, PT], F32, tag="ps4", name=f"ps4_{t}_{mg}")
                        for mi in range(2):
                            m = mg * 2 + mi
                            for k in range(4):
                                nc.tensor.matmul(
                                    ps4[:, mi, :], w4_sb[:, k, m, :], h3_sb[:, k, :],
                                    start=(k == 0), stop=(k == 3),
                                )
                        nc.vector.tensor_reduce(
                            out=Mx4_sb[:, 2 * mg : 2 * mg + 2, t : t + 1], in_=ps4, axis=AXX, op=ALU_MAX
                        )

                # prologue (tiles 0/1 were produced during assembly)
                emit_L3(0)
                # steady state
                for t in range(Tm):
                    if t + 1 < Tm:
                        emit_L3(t + 1)
                    h1_n = emit_L1(t + 2) if t + 2 < Tm else None
                    emit_L4(t)
                    if h1_n is not None:
                        emit_L2(t + 2, h1_n)

            nc.sync.dma_start(out=mx4.ap(), in_=Mx4_sb)

    nc.finalize()
    return nc


def _prepare(x, seg_ids, B):
    """Pad + pack segments into per-core macro-tile streams."""
    counts = np.bincount(seg_ids, minlength=B)
    starts = np.concatenate([[0], np.cumsum(counts)])
    seg_tiles = [(int(c) + PT - 1) // PT for c in counts]

    SLOTS = (B + N_CORES - 1) // N_CORES
    order = np.argsort(-np.asarray(seg_tiles), kind="stable")
    core_segs: list[list[int]] = [[] for _ in range(N_CORES)]
    core_load = [0] * N_CORES
    for s in order:
        cands = [c for c in range(N_CORES) if len(core_segs[c]) < SLOTS]
        c = min(cands, key=lambda i: core_load[i])
        core_segs[c].append(int(s))
        core_load[c] += seg_tiles[s]

    # local search: swap segments between cores to shave the max load
    ideal = (sum(seg_tiles) + N_CORES - 1) // N_CORES
    for _ in range(200):
        if max(core_load) <= ideal:
            break
        hi = max(range(N_CORES), key=lambda i: core_load[i])
        improved = False
        for lo in sorted(range(N_CORES), key=lambda i: core_load[i]):
            if lo == hi:
                continue
            for ia, sa in enumerate(core_segs[hi]):
                for ib, sb in enumerate(core_segs[lo]):
                    d = seg_tiles[sa] - seg_tiles[sb]
                    if d > 0 and max(core_load[hi] - d, core_load[lo] + d) < max(
                        core_load[hi], core_load[lo]
                    ):
                        core_segs[hi][ia], core_segs[lo][ib] = sb, sa
                        core_load[hi] -= d
                        core_load[lo] += d
                        improved = True
                        break
                if improved:
                    break
            if improved:
                break
        if not improved:
            break
    Tm = max(core_load)

    xT_cores, mask_cores, post = [], [], []
    for c in range(N_CORES):
        pts_list, slot_of_tile = [], []
        for slot, s in enumerate(core_segs[c]):
            seg_pts = x[starts[s] : starts[s + 1]]
            ntile = seg_tiles[s]
            padn = ntile * PT - len(seg_pts)
            if padn:
                seg_pts = np.concatenate([seg_pts, seg_pts[:padn]])
            pts_list.append(seg_pts)
            slot_of_tile += [slot] * ntile
        extra = Tm - core_load[c]
        if extra:
            pts_list.append(np.tile(pts_list[0][:PT], (extra, 1)))
            slot_of_tile += [0] * extra
        xc = np.concatenate(pts_list).astype(np.float16)
        xT_cores.append(np.ascontiguousarray(xc.T))
        sot = np.asarray(slot_of_tile)
        Tmp = Tm + (Tm % 2)
        m01 = np.zeros((SLOTS, Tmp), np.float32)
        m01[:, :Tm] = sot[None, :] == np.arange(SLOTS)[:, None]
        mask_cores.append(np.ascontiguousarray(np.broadcast_to(m01[None], (128, SLOTS, Tmp))))
        post.append((core_segs[c], sot))
    return Tm, SLOTS, xT_cores, mask_cores, post


def make_in_maps(inputs):
    """Fold BN, pack points, and build the per-core SPMD input dicts.

    Returns (key, in_maps, post, b4f) where key indexes _PROGRAM_CACHE.
    """
    x = np.asarray(inputs["x"], np.float32)
    seg_ids = np.asarray(inputs["seg_ids"])
    B = int(inputs["num_segments"])

    Wf, bf = [], []
    for i in (1, 2, 3, 4):
        W = np.asarray(inputs[f"W{i}"], np.float32)
        b = np.asarray(inputs[f"b{i}"], np.float32)
        ga = np.asarray(inputs[f"g{i}"], np.float32)
        be = np.asarray(inputs[f"be{i}"], np.float32)
        m = np.asarray(inputs[f"m{i}"], np.float32)
        v = np.asarray(inputs[f"v{i}"], np.float32)
        sc = ga / np.sqrt(v + EPS)
        Wf.append(np.ascontiguousarray(W * sc[None, :]))
        bf.append((b - m) * sc + be)
    W1f, W2f, W3f, W4f = Wf
    b1f, b2f, b3f, b4f = bf

    Tm, SLOTS, xT_cores, mask_cores, post = _prepare(x, seg_ids, B)

    w1d = W1f.astype(np.float16)
    w2d = np.ascontiguousarray(W2f.reshape(128, 2, 128).astype(np.float16))
    w3ad = np.ascontiguousarray(W3f[:256].reshape(2, 128, 4, 128).transpose(1, 0, 2, 3).astype(np.float16))
    w3bd = np.ascontiguousarray(W3f[256:].reshape(2, 128, 4, 128).transpose(1, 0, 2, 3).astype(np.float16))
    w4d = np.ascontiguousarray(W4f.reshape(4, 128, 8, 128).transpose(1, 0, 2, 3).astype(np.float16))
    b1d = np.ascontiguousarray(b1f.reshape(128, 1))
    b2d = np.ascontiguousarray(b2f.reshape(2, 128).T)
    b3d = np.ascontiguousarray(b3f.reshape(4, 128).T)

    in_maps = [
        {
            "xT": xT_cores[c],
            "mask": mask_cores[c],
            "w1": w1d,
            "w2": w2d,
            "w3a": w3ad,
            "w3b": w3bd,
            "w4": w4d,
            "b1": b1d,
            "b2": b2d,
            "b3": b3d,
        }
        for c in range(N_CORES)
    ]
    return (Tm, SLOTS), in_maps, post, b4f


def postprocess(results, post, b4f, B):
    out = np.zeros((B, 1024), np.float32)
    for c in range(N_CORES):
        mx4 = results[c]["mx4"]  # [128, 8, Tm]
        segs, sot = post[c]
        for slot, s in enumerate(segs):
            cols = np.flatnonzero(sot == slot)
            raw = mx4[:, :, cols].max(axis=2)  # [128, 8]
            out[s] = np.maximum(raw.T.reshape(1024) + b4f, 0.0)
    return out


def get_program(key):
    if key not in _PROGRAM_CACHE:
        _PROGRAM_CACHE[key] = _build_program(*key)
    return _PROGRAM_CACHE[key]


def kernel(**inputs) -> np.ndarray:
    B = int(inputs["num_segments"])
    key, in_maps, post, b4f = make_in_maps(inputs)
    nc = get_program(key)
    res = run_bass_kernel_spmd(nc, in_maps, core_ids=list(range(N_CORES)))
    return postprocess(res.results, post, b4f, B)
